# revision 39
# baseline (speedup 1.0000x reference)
"""Trainium2 Bass kernel for GroupLinear:
    out = einsum('lgi,lgj,ogij->lo', x1, x2, W.reshape(O,g,b,b)) + bias

Equivalent to Y = outer @ W.T + b where outer[l, k] (k = g*b*b + i*b + j) is
the blockwise outer product x1[l,g,i]*x2[l,g,j] -- a [2048, 65536] @
[65536, 1024] matmul whose LHS is generated on the fly.

Sharding: tensor-parallel over the contraction dim. Core c owns weight
blocks g in {2c, 2c+1} (K_local = 8192), computes a full [2048, 1024] fp32
partial, and the host sums the 8 partials (+ bias).

The PE moving-operand port (2 B/cycle/partition) is the hard floor: W
streams through the PE once per 128-token l-block, so cycles =
1.048M * (1 - f/2) where f is the fp8 fraction.  bf16 (1 elem/cycle) and
fp8 DoubleRow (2 elem/cycle) both saturate the port; fp8 halves the
cycles for its chunks.  f is precision-capped: e4m3 RNE on both T and W
gives ~3.57e-2*sqrt(f) output rel err, so f=18/64 lands at 1.907e-2
(measured on hw) against the 2e-2 gate.

v3 scheme (v2 measured 413.8us; v1 pure-bf16 ~495us):
  * NFP8=18 (was 16): -6.5us of PE streaming.
  * Dual DMA queues: the W stream owns the SP (sync) HWDGE queue; x2 +
    x1-slab loads go on the Activation HWDGE queue, so the slab/x2
    dispatches no longer delay W chunk 0 (first matmul was at t=18.5us,
    gated by W c0 queued behind 2.5 MB of slab traffic).
  * Slab-0 quarter 0 split into a 4-chunk head piece so the first DVE
    outer product (and thus the first matmul) needs only 128 KB of x1.
  * Per-half drains + output DMAs: each psum half drains and DMAs as it
    finishes, so the end-of-kernel tail is one 0.25 MB half (split
    across both queues for the final l-block) instead of 0.5 MB.

Device loop (per core):
  startup (lb 0..3, chunk-major over 8 psum banks):
    DVE: outer products per (piece/quarter)-slab as the slab DMA lands
    PE : for each W chunk c: 8 accumulating matmuls (4 lb x 2 halves)
  steady (lb 4..15, lblock-major):
    DVE: bf16 in-place muls + fp8 muls into t8 tile
    PE : per half: NBF bf16 matmuls + NFP8/2 fp8 DoubleRow matmuls
  drain: ACT copies psum->sbuf per half with 2^-12 rescale, DMA out.

All of W is pre-scaled by 2^12 on the host so the e4m3 W chunks sit in
the normal range; bf16 chunks share the same scale so both accumulate
into the same psum group, and the psum->sbuf drain rescales by 2^-12.

The repeat/unroll knobs exist only for wall-clock benchmarking (repeat
wraps the body in a hardware For_i loop; unroll stacks extra bodies per
iteration with W/x2 loaded only once).
"""

import sys
import os
import numpy as np

sys.path.insert(0, "/opt/trn_rl_repo")

import ml_dtypes  # noqa: E402

BF16 = ml_dtypes.bfloat16
F8E4 = ml_dtypes.float8_e4m3

L = 2048
H = 1024
O = 1024
B = 64
G = 16
NCORES = 8
GPC = G // NCORES          # weight blocks per core = 2
KL = GPC * B * B           # local contraction dim = 8192
NCHUNK = KL // 128         # 64 k-chunks of 128
LB = 128                   # l-block (tokens per psum tile)
NLB = L // LB              # 16
SLB = int(os.environ.get("GL_SLB", "4"))  # startup l-blocks, chunk-major
WSCALE = 2.0 ** 12         # host-side W pre-scale (exact in bf16)
PA = 4                     # head-piece chunks of slab quarter 0

NFP8 = int(os.environ.get("GL_NFP8", "18"))   # trailing fp8 chunks (even)
NBF = NCHUNK - NFP8

# timing ablations (wrong numerics, benchmarking only):
#   'nodma' = steady state reuses startup slabs, no per-lb slab DMA / DVE
#   'nope'  = no matmuls / drains / out DMA anywhere
#   'nomul' = no DVE outer-product muls (PE reads raw slabs)
ABLATE = os.environ.get("GL_ABLATE", "")

_cache = {}


def _build_nc(repeat=1, unroll=1):
    from concourse import bass, tile, bacc
    from contextlib import nullcontext

    mybir = bass.mybir
    bf = mybir.dt.bfloat16
    f8 = mybir.dt.float8e4
    f32 = mybir.dt.float32
    DR = mybir.MatmulPerfMode.DoubleRow

    assert NFP8 % 2 == 0 and 0 <= NFP8 < NCHUNK
    assert NBF >= 32, "fp8 chunks must all fall in the second g block"

    nc = bacc.Bacc("TRN2", target_bir_lowering=False, debug=False)
    wbf = nc.dram_tensor("wbf", [128, NBF * O], bf, kind="ExternalInput")
    if NFP8:
        wf8 = nc.dram_tensor("wf8", [128, NFP8 * O], f8, kind="ExternalInput")
    x1r = nc.dram_tensor("x1r", [NLB, 128, KL], bf, kind="ExternalInput")
    x2s = nc.dram_tensor("x2s", [128, GPC, L], bf, kind="ExternalInput")
    # bf16 partials: the host sum adds ~0.2% quantization to each 1/8
    # partial, negligible vs the fp8-path error; halves the out traffic
    out = nc.dram_tensor("out", [L, O], bf, kind="ExternalOutput")

    QL = KL // 4  # quarter-slab length (16 chunks)

    with tile.TileContext(nc) as tc:
        with (
            tc.tile_pool(name="wbfp", bufs=1) as wbfp,
            tc.tile_pool(name="wf8p", bufs=1) as wf8p,
            tc.tile_pool(name="x2pool", bufs=1) as x2pool,
            tc.tile_pool(name="xpool", bufs=max(SLB, 3)) as xpool,
            tc.tile_pool(name="t8pool", bufs=max(SLB, 2) + 1) as t8pool,
            tc.tile_pool(name="opool", bufs=2) as opool,
            tc.tile_pool(name="wupool", bufs=1) as wupool,
            tc.tile_pool(name="psum", bufs=8, space="PSUM") as psum,
            tc.For_i(0, repeat, 1) if repeat > 1 else nullcontext(),
        ):
            wbft = wbfp.tile([128, NBF * O], bf)
            wf8t = (
                wf8p.tile([128, NFP8 * O], f8, name="wf8t", tag="wf8t")
                if NFP8 else None
            )
            x2c = x2pool.tile([128, GPC * L], bf, name="x2c", tag="x2c")
            x2r = x2c[:].rearrange("p (g l) -> p g l", g=GPC)
            wf8r = (
                wf8t[:].rearrange("q (cf o) -> q cf o", o=O) if NFP8 else None
            )

            def dr_lhs(t8_tile, p):
                return t8_tile[:, (2 * p) * 128:(2 * p + 2) * 128].rearrange(
                    "q (two m) -> q two m", two=2
                )

            def emit_body(first):
                xts = [
                    xpool.tile([128, KL], bf, name="xt", tag="xt")
                    for _ in range(SLB)
                ]
                use_f8 = NFP8 and ABLATE != "nomul"
                t8s = (
                    [
                        t8pool.tile([128, NFP8 * 128], f8, name="t8", tag="t8")
                        for _ in range(SLB)
                    ]
                    if use_f8 else [None] * SLB
                )

                # ---- DMA schedule ----
                # sync queue: W stream (chunk 0 first, then growing
                # batches), then the fp8 W halves.  Startup x2 + slab
                # loads ride the Activation HWDGE queue in parallel, so
                # the first matmul only waits for W c0 + a 128 KB slab
                # head piece + the lb0 x2 columns.
                # Per-ring DMA bandwidth is ~40 GB/s and each queue's
                # successive DMAs land on successive rings, so the early
                # startup splits everything small: W c0 across 2 rings,
                # c1..c8 as singles (one ring each, 0.6us dispatch
                # cadence), pairs/fours later once the PE's 1.73us/chunk
                # consumption dominates.
                # Each HWDGE queue has only ~4 rings at ~40 GB/s with
                # ~2us fixed latency per transfer, so fine-grained
                # splitting backfires; the startup instead runs warm-up
                # dummies on the PE until the stream is comfortably
                # supplied (~15us) and then never stalls.
                if first:
                    for half in range(4):
                        nc.sync.dma_start(
                            wbft[:, half * 512:(half + 1) * 512],
                            wbf[:, half * 512:(half + 1) * 512],
                        )
                    for c in range(2, 9):
                        nc.sync.dma_start(
                            wbft[:, c * O:(c + 1) * O], wbf[:, c * O:(c + 1) * O]
                        )
                if first and SLB:
                    # x2 startup columns: 4 transfers on the gpsimd SWDGE
                    # queue -- per-g 256-col pieces so lb0/lb1 unblock
                    # first
                    for g in range(GPC):
                        nc.gpsimd.dma_start(
                            x2r[:, g, 0:2 * LB], x2s[:, g, 0:2 * LB]
                        )
                    for g in range(GPC):
                        nc.gpsimd.dma_start(
                            x2r[:, g, 2 * LB:SLB * LB], x2s[:, g, 2 * LB:SLB * LB]
                        )
                for lb in range(SLB):
                    nc.scalar.dma_start(
                        xts[lb][:, 0:PA * 128], x1r[lb][:, 0:PA * 128]
                    )
                # piece-b in two halves per l-block: chunks 4..10, 10..16
                PB = 10
                for lb in range(SLB):
                    nc.scalar.dma_start(
                        xts[lb][:, PA * 128:PB * 128], x1r[lb][:, PA * 128:PB * 128]
                    )
                if first:
                    for c in range(9, 17, 2):
                        nc.sync.dma_start(
                            wbft[:, c * O:(c + 2) * O], wbf[:, c * O:(c + 2) * O]
                        )
                for lb in range(SLB):
                    nc.scalar.dma_start(
                        xts[lb][:, PB * 128:QL], x1r[lb][:, PB * 128:QL]
                    )
                if first:
                    for c in range(17, NBF, 4):
                        ce = min(c + 4, NBF)
                        nc.sync.dma_start(
                            wbft[:, c * O:ce * O], wbf[:, c * O:ce * O]
                        )
                for q in range(1, 4):
                    for lb in range(SLB):
                        nc.scalar.dma_start(
                            xts[lb][:, q * QL:(q + 1) * QL],
                            x1r[lb][:, q * QL:(q + 1) * QL],
                        )
                if first:
                    if NFP8:
                        hw8 = NFP8 * O // 2
                        nc.sync.dma_start(wf8t[:, 0:hw8], wf8[:, 0:hw8])
                        nc.sync.dma_start(wf8t[:, hw8:], wf8[:, hw8:])
                    # x2 tails (needed only from lb=SLB on) go LAST so
                    # their 3 MB doesn't steal HBM bandwidth from the W
                    # stream during startup
                    nc.scalar.dma_start(
                        x2r[:, :, SLB * LB:L], x2s[:, :, SLB * LB:L]
                    )

                # ---- DVE: startup outer products ----
                # One broadcast mul covers a whole run of chunks sharing the
                # same g (x2 slice): in1 = x2[128,128] broadcast over chunks.
                def bcast_mul(xt_tile, t8_tile, lsl, c0, c1):
                    """outer products for chunks [c0, c1) of one l-block."""
                    if ABLATE == "nomul":
                        return
                    for g in range(GPC):
                        glo, ghi = max(c0, g * 32), min(c1, (g + 1) * 32, NBF)
                        if glo < ghi:
                            seg = xt_tile[:, glo * 128:ghi * 128].rearrange(
                                "p (c l) -> p c l", c=ghi - glo
                            )
                            bc = x2r[:, g, lsl].rearrange(
                                "p (o l) -> p o l", o=1
                            ).to_broadcast([128, ghi - glo, LB])
                            nc.vector.tensor_mul(seg, seg, bc)
                    if NFP8:
                        flo, fhi = max(c0, NBF), c1
                        if flo < fhi:
                            g = flo >> 5
                            seg8 = t8_tile[
                                :, (flo - NBF) * 128:(fhi - NBF) * 128
                            ].rearrange("p (c l) -> p c l", c=fhi - flo)
                            src = xt_tile[:, flo * 128:fhi * 128].rearrange(
                                "p (c l) -> p c l", c=fhi - flo
                            )
                            bc = x2r[:, g, lsl].rearrange(
                                "p (o l) -> p o l", o=1
                            ).to_broadcast([128, fhi - flo, LB])
                            nc.vector.tensor_mul(seg8, src, bc)

                # DVE mul order mirrors DMA arrival (the DVE queue is
                # in-order): chunk-0 for all l-blocks, rest of piece-a,
                # each piece-b half, then quarters 1..3.
                def q0_sl(lb):
                    return slice(lb * LB, (lb + 1) * LB)

                for lb in range(SLB):
                    bcast_mul(xts[lb], t8s[lb], q0_sl(lb), 0, PA)
                for lb in range(SLB):
                    bcast_mul(xts[lb], t8s[lb], q0_sl(lb), PA, PB)
                for lb in range(SLB):
                    bcast_mul(xts[lb], t8s[lb], q0_sl(lb), PB, 16)
                for q in range(1, 4):
                    for lb in range(SLB):
                        bcast_mul(xts[lb], t8s[lb], q0_sl(lb), q * 16, (q + 1) * 16)

                # ---- PE: startup, chunk-major across 8 psum banks ----
                if ABLATE == "nope":
                    # DMA+DVE only: emit steady slab loads + muls, no PE/out
                    for lb in range(SLB, NLB):
                        xt = xpool.tile([128, KL], bf, name="xt", tag="xt")
                        nc.scalar.dma_start(xt[:], x1r[lb])
                        t8 = (
                            t8pool.tile(
                                [128, NFP8 * 128], f8, name="t8", tag="t8"
                            )
                            if NFP8 else None
                        )
                        bcast_mul(
                            xt, t8, slice(lb * LB, (lb + 1) * LB), 0, NCHUNK
                        )
                    return
                # PE warm-up: self-contained dummy matmuls on a memset
                # scratch tile while the first W chunk + slab pieces are
                # still in flight, so the PE exits its low/mid p-states
                # (427ns/instr for the first ~3us otherwise) before real
                # work arrives.  The warm psum tile shares a bank with a
                # later pss tile; WAW ordering covers it.
                if first and ABLATE != "nope":
                    warm = wupool.tile([128, 128], bf, name="warm", tag="warm")
                    nc.vector.memset(warm[:], 0.0)
                    psw = psum.tile([128, 512], f32, name="pss", tag="ps")
                    for _ in range(int(os.environ.get("GL_WARM", "100"))):
                        nc.tensor.matmul(
                            psw[:, 0:128], warm[:], warm[:],
                            start=True, stop=True,
                        )

                pss = [
                    psum.tile([128, 512], f32, name="pss", tag="ps")
                    for _ in range(2 * SLB)
                ]

                def su_mm(c, lb, h):
                    nc.tensor.matmul(
                        pss[lb * 2 + h][:],
                        xts[lb][:, c * 128:(c + 1) * 128],
                        wbft[:, c * O + h * 512:c * O + h * 512 + 512],
                        start=(c == 0),
                        stop=(not use_f8 and c == NBF - 1),
                    )

                # chunk-major across all startup l-blocks: the W stream
                # supplies ~1 chunk per 0.6us dispatch slot while the PE
                # consumes one per 1.73us -- consuming any faster (e.g. a
                # single-lb bridge) just starves on W.  h in the middle:
                # each chunk's h0 matmuls only need the chunk's first 512
                # W columns (c0/c1 are DMA'd as 512-col halves).
                for c in range(NBF):
                    for h in range(2):
                        for lb in range(SLB):
                            su_mm(c, lb, h)
                for p in range(NFP8 // 2 if use_f8 else 0):
                    for lb in range(SLB):
                        for h in range(2):
                            nc.tensor.matmul(
                                pss[lb * 2 + h][:],
                                dr_lhs(t8s[lb], p),
                                wf8r[:, 2 * p:2 * p + 2, h * 512:h * 512 + 512],
                                start=False,
                                stop=(p == NFP8 // 2 - 1),
                                perf_mode=DR,
                            )
                for lb in range(SLB):
                    lsl = slice(lb * LB, (lb + 1) * LB)
                    ot = opool.tile([128, O], bf, name="ot", tag="ot")
                    for h in range(2):
                        hs = slice(h * 512, h * 512 + 512)
                        nc.scalar.mul(ot[:, hs], pss[lb * 2 + h][:], 1.0 / WSCALE)
                        nc.sync.dma_start(out[lsl, hs], ot[:, hs])

                # ---- steady state: lblock-major ----
                for lb in range(SLB, NLB):
                    last = lb == NLB - 1
                    if ABLATE == "nodma":
                        xt, t8 = xts[lb % SLB], t8s[lb % SLB]
                    else:
                        xt = xpool.tile([128, KL], bf, name="xt", tag="xt")
                        nc.scalar.dma_start(xt[:], x1r[lb])
                        t8 = (
                            t8pool.tile([128, NFP8 * 128], f8, name="t8", tag="t8")
                            if use_f8 else None
                        )
                    lsl = slice(lb * LB, (lb + 1) * LB)
                    if ABLATE != "nodma":
                        bcast_mul(xt, t8, lsl, 0, NCHUNK)
                    ot = opool.tile([128, O], bf, name="ot", tag="ot")
                    for h in range(2):
                        hs = slice(h * 512, h * 512 + 512)
                        ps = psum.tile([128, 512], f32, name="pss", tag="ps")
                        for c in range(NBF):
                            nc.tensor.matmul(
                                ps[:],
                                xt[:, c * 128:(c + 1) * 128],
                                wbft[:, c * O + h * 512:c * O + h * 512 + 512],
                                start=(c == 0),
                                stop=(not use_f8 and c == NBF - 1),
                            )
                        for p in range(NFP8 // 2 if use_f8 else 0):
                            nc.tensor.matmul(
                                ps[:],
                                dr_lhs(t8, p),
                                wf8r[:, 2 * p:2 * p + 2, h * 512:h * 512 + 512],
                                start=False,
                                stop=(p == NFP8 // 2 - 1),
                                perf_mode=DR,
                            )
                        nc.scalar.mul(ot[:, hs], ps[:], 1.0 / WSCALE)
                        if last and h == 1:
                            # final transfer: quarter the tail across 2
                            # rings on each HWDGE queue
                            l0 = lb * LB
                            for p0, eng in (
                                (0, nc.sync), (64, nc.scalar),
                                (32, nc.sync), (96, nc.scalar),
                            ):
                                eng.dma_start(
                                    out[l0 + p0:l0 + p0 + 32, hs],
                                    ot[p0:p0 + 32, hs],
                                )
                        else:
                            nc.sync.dma_start(out[lsl, hs], ot[:, hs])

            for _u in range(unroll):
                emit_body(_u == 0)

    nc.compile()
    return nc


def _prep_inputs(input1, input2, W):
    """Host-side shard + layout (transposes / gathers / dtype casts only)."""
    x1 = np.ascontiguousarray(input1, dtype=np.float32)
    x2 = np.ascontiguousarray(input2, dtype=np.float32)
    Wt = np.ascontiguousarray(W.T, dtype=np.float32) * np.float32(WSCALE)

    in_maps = []
    for core in range(NCORES):
        ks = slice(core * KL, (core + 1) * KL)
        gs = slice(core * GPC, (core + 1) * GPC)
        # weights: [k_local, o] -> [c, p, o] -> [p, c, o] (chunk-major free dim)
        wchunks = (
            Wt[ks].reshape(NCHUNK, 128, O).transpose(1, 0, 2)
        )  # [128, NCHUNK, O] fp32 (scaled)
        wbf = np.ascontiguousarray(
            wchunks[:, :NBF, :].reshape(128, NBF * O).astype(BF16)
        )
        wf8 = np.ascontiguousarray(
            wchunks[:, NBF:, :].reshape(128, NFP8 * O).astype(F8E4)
        )
        # x1 replicated over j: k_local = g*B*B + i*B + j -> x1[l, g, i]
        x1g = x1.reshape(L, G, B)[:, gs, :].transpose(1, 2, 0)  # [g, i, l]
        rep = np.repeat(x1g, B, axis=1).reshape(KL, L)          # [k_local, l]
        x1rc = (
            rep.reshape(NCHUNK, 128, NLB, LB)
            .transpose(2, 1, 0, 3)
            .reshape(NLB, 128, KL)
            .astype(BF16)
        )
        # x2 stacked twice along partitions: row p -> j = p % 64
        x2g = x2.reshape(L, G, B)[:, gs, :].transpose(1, 2, 0)  # [g, j, l]
        x2st = np.concatenate([x2g, x2g], axis=1).astype(BF16)  # [g, 128, l]
        x2st = x2st.transpose(1, 0, 2)                          # [128, g, l]
        m = {
            "wbf": wbf,
            "x1r": np.ascontiguousarray(x1rc),
            "x2s": np.ascontiguousarray(x2st),
        }
        if NFP8:
            m["wf8"] = wf8
        in_maps.append(m)
    return in_maps


def run(input1, input2, W, b, trace=False):
    """Shard, run on 8 NeuronCores, unshard. Returns (out, BassKernelResults)."""
    from concourse.bass_utils import run_bass_kernel_spmd

    if "nc" not in _cache:
        _cache["nc"] = _build_nc()
    nc = _cache["nc"]

    in_maps = _prep_inputs(input1, input2, W)
    res = run_bass_kernel_spmd(
        nc, in_maps, list(range(NCORES)), trace=trace
    )
    acc = np.zeros((L, O), dtype=np.float32)
    for core in range(NCORES):
        acc += res.results[core]["out"]
    acc += np.asarray(b, dtype=np.float32)[None, :]
    return acc, res


def kernel(input1, input2, W, b):
    out, _ = run(input1, input2, W, b, trace=False)
    return out


if __name__ == "__main__":
    rng = np.random.default_rng(0)
    x1 = rng.standard_normal((L, H), dtype=np.float32)
    x2 = rng.standard_normal((L, H), dtype=np.float32)
    W = rng.standard_normal((O, H * B), dtype=np.float32) / 256.0
    b = rng.standard_normal((O,), dtype=np.float32) / 256.0
    out = kernel(x1, x2, W, b)
    print("out", out.shape, out.dtype, float(np.abs(out).max()))


# revision 40
# speedup vs baseline: 1.0028x; 1.0028x over previous
"""Trainium2 Bass kernel for GroupLinear:
    out = einsum('lgi,lgj,ogij->lo', x1, x2, W.reshape(O,g,b,b)) + bias

Equivalent to Y = outer @ W.T + b where outer[l, k] (k = g*b*b + i*b + j) is
the blockwise outer product x1[l,g,i]*x2[l,g,j] -- a [2048, 65536] @
[65536, 1024] matmul whose LHS is generated on the fly.

Sharding: tensor-parallel over the contraction dim. Core c owns weight
blocks g in {2c, 2c+1} (K_local = 8192), computes a full [2048, 1024] fp32
partial, and the host sums the 8 partials (+ bias).

The PE moving-operand port (2 B/cycle/partition) is the hard floor: W
streams through the PE once per 128-token l-block, so cycles =
1.048M * (1 - f/2) where f is the fp8 fraction.  bf16 (1 elem/cycle) and
fp8 DoubleRow (2 elem/cycle) both saturate the port; fp8 halves the
cycles for its chunks.  f is precision-capped: e4m3 RNE on both T and W
gives ~3.57e-2*sqrt(f) output rel err, so f=18/64 lands at 1.907e-2
(measured on hw) against the 2e-2 gate.

v3 scheme (v2 measured 413.8us; v1 pure-bf16 ~495us):
  * NFP8=18 (was 16): -6.5us of PE streaming.
  * Dual DMA queues: the W stream owns the SP (sync) HWDGE queue; x2 +
    x1-slab loads go on the Activation HWDGE queue, so the slab/x2
    dispatches no longer delay W chunk 0 (first matmul was at t=18.5us,
    gated by W c0 queued behind 2.5 MB of slab traffic).
  * Slab-0 quarter 0 split into a 4-chunk head piece so the first DVE
    outer product (and thus the first matmul) needs only 128 KB of x1.
  * Per-half drains + output DMAs: each psum half drains and DMAs as it
    finishes, so the end-of-kernel tail is one 0.25 MB half (split
    across both queues for the final l-block) instead of 0.5 MB.

Device loop (per core):
  startup (lb 0..3, chunk-major over 8 psum banks):
    DVE: outer products per (piece/quarter)-slab as the slab DMA lands
    PE : for each W chunk c: 8 accumulating matmuls (4 lb x 2 halves)
  steady (lb 4..15, lblock-major):
    DVE: bf16 in-place muls + fp8 muls into t8 tile
    PE : per half: NBF bf16 matmuls + NFP8/2 fp8 DoubleRow matmuls
  drain: ACT copies psum->sbuf per half with 2^-12 rescale, DMA out.

All of W is pre-scaled by 2^12 on the host so the e4m3 W chunks sit in
the normal range; bf16 chunks share the same scale so both accumulate
into the same psum group, and the psum->sbuf drain rescales by 2^-12.

The repeat/unroll knobs exist only for wall-clock benchmarking (repeat
wraps the body in a hardware For_i loop; unroll stacks extra bodies per
iteration with W/x2 loaded only once).
"""

import sys
import os
import numpy as np

sys.path.insert(0, "/opt/trn_rl_repo")

import ml_dtypes  # noqa: E402

BF16 = ml_dtypes.bfloat16
F8E4 = ml_dtypes.float8_e4m3

L = 2048
H = 1024
O = 1024
B = 64
G = 16
NCORES = 8
GPC = G // NCORES          # weight blocks per core = 2
KL = GPC * B * B           # local contraction dim = 8192
NCHUNK = KL // 128         # 64 k-chunks of 128
LB = 128                   # l-block (tokens per psum tile)
NLB = L // LB              # 16
SLB = int(os.environ.get("GL_SLB", "4"))  # startup l-blocks, chunk-major
WSCALE = 2.0 ** 12         # host-side W pre-scale (exact in bf16)
PA = 4                     # head-piece chunks of slab quarter 0

NFP8 = int(os.environ.get("GL_NFP8", "18"))   # trailing fp8 chunks (even)
NBF = NCHUNK - NFP8

# timing ablations (wrong numerics, benchmarking only):
#   'nodma' = steady state reuses startup slabs, no per-lb slab DMA / DVE
#   'nope'  = no matmuls / drains / out DMA anywhere
#   'nomul' = no DVE outer-product muls (PE reads raw slabs)
ABLATE = os.environ.get("GL_ABLATE", "")

_cache = {}


def _build_nc(repeat=1, unroll=1):
    from concourse import bass, tile, bacc
    from contextlib import nullcontext

    mybir = bass.mybir
    bf = mybir.dt.bfloat16
    f8 = mybir.dt.float8e4
    f32 = mybir.dt.float32
    DR = mybir.MatmulPerfMode.DoubleRow

    assert NFP8 % 2 == 0 and 0 <= NFP8 < NCHUNK
    assert NBF >= 32, "fp8 chunks must all fall in the second g block"

    nc = bacc.Bacc("TRN2", target_bir_lowering=False, debug=False)
    wbf = nc.dram_tensor("wbf", [128, NBF * O], bf, kind="ExternalInput")
    if NFP8:
        wf8 = nc.dram_tensor("wf8", [128, NFP8 * O], f8, kind="ExternalInput")
    x1r = nc.dram_tensor("x1r", [NLB, 128, KL], bf, kind="ExternalInput")
    x2s = nc.dram_tensor("x2s", [128, GPC, L], bf, kind="ExternalInput")
    # bf16 partials: the host sum adds ~0.2% quantization to each 1/8
    # partial, negligible vs the fp8-path error; halves the out traffic
    out = nc.dram_tensor("out", [L, O], bf, kind="ExternalOutput")

    QL = KL // 4  # quarter-slab length (16 chunks)

    with tile.TileContext(nc) as tc:
        with (
            tc.tile_pool(name="wbfp", bufs=1) as wbfp,
            tc.tile_pool(name="wf8p", bufs=1) as wf8p,
            tc.tile_pool(name="x2pool", bufs=1) as x2pool,
            tc.tile_pool(name="xpool", bufs=max(SLB, 3)) as xpool,
            tc.tile_pool(name="t8pool", bufs=max(SLB, 2) + 1) as t8pool,
            tc.tile_pool(name="opool", bufs=2) as opool,
            tc.tile_pool(name="wupool", bufs=1) as wupool,
            tc.tile_pool(name="psum", bufs=8, space="PSUM") as psum,
            tc.For_i(0, repeat, 1) if repeat > 1 else nullcontext(),
        ):
            wbft = wbfp.tile([128, NBF * O], bf)
            wf8t = (
                wf8p.tile([128, NFP8 * O], f8, name="wf8t", tag="wf8t")
                if NFP8 else None
            )
            x2c = x2pool.tile([128, GPC * L], bf, name="x2c", tag="x2c")
            x2r = x2c[:].rearrange("p (g l) -> p g l", g=GPC)
            wf8r = (
                wf8t[:].rearrange("q (cf o) -> q cf o", o=O) if NFP8 else None
            )

            def dr_lhs(t8_tile, p):
                return t8_tile[:, (2 * p) * 128:(2 * p + 2) * 128].rearrange(
                    "q (two m) -> q two m", two=2
                )

            def emit_body(first):
                xts = [
                    xpool.tile([128, KL], bf, name="xt", tag="xt")
                    for _ in range(SLB)
                ]
                use_f8 = NFP8 and ABLATE != "nomul"
                t8s = (
                    [
                        t8pool.tile([128, NFP8 * 128], f8, name="t8", tag="t8")
                        for _ in range(SLB)
                    ]
                    if use_f8 else [None] * SLB
                )

                # ---- DMA schedule ----
                # sync queue: W stream (chunk 0 first, then growing
                # batches), then the fp8 W halves.  Startup x2 + slab
                # loads ride the Activation HWDGE queue in parallel, so
                # the first matmul only waits for W c0 + a 128 KB slab
                # head piece + the lb0 x2 columns.
                # Per-ring DMA bandwidth is ~40 GB/s and each queue's
                # successive DMAs land on successive rings, so the early
                # startup splits everything small: W c0 across 2 rings,
                # c1..c8 as singles (one ring each, 0.6us dispatch
                # cadence), pairs/fours later once the PE's 1.73us/chunk
                # consumption dominates.
                # Each HWDGE queue has only ~4 rings at ~40 GB/s with
                # ~2us fixed latency per transfer, so fine-grained
                # splitting backfires; the startup instead runs warm-up
                # dummies on the PE until the stream is comfortably
                # supplied (~15us) and then never stalls.
                if first:
                    for half in range(4):
                        nc.sync.dma_start(
                            wbft[:, half * 512:(half + 1) * 512],
                            wbf[:, half * 512:(half + 1) * 512],
                        )
                    for c in range(2, 9):
                        nc.sync.dma_start(
                            wbft[:, c * O:(c + 1) * O], wbf[:, c * O:(c + 1) * O]
                        )
                if first and SLB:
                    # x2 startup columns: 4 transfers on the gpsimd SWDGE
                    # queue -- per-g 256-col pieces so lb0/lb1 unblock
                    # first
                    for g in range(GPC):
                        nc.gpsimd.dma_start(
                            x2r[:, g, 0:2 * LB], x2s[:, g, 0:2 * LB]
                        )
                    for g in range(GPC):
                        nc.gpsimd.dma_start(
                            x2r[:, g, 2 * LB:SLB * LB], x2s[:, g, 2 * LB:SLB * LB]
                        )
                for lb in range(SLB):
                    nc.scalar.dma_start(
                        xts[lb][:, 0:PA * 128], x1r[lb][:, 0:PA * 128]
                    )
                # piece-b in two halves per l-block: chunks 4..10, 10..16
                PB = 10
                for lb in range(SLB):
                    nc.scalar.dma_start(
                        xts[lb][:, PA * 128:PB * 128], x1r[lb][:, PA * 128:PB * 128]
                    )
                if first:
                    for c in range(9, 17, 2):
                        nc.sync.dma_start(
                            wbft[:, c * O:(c + 2) * O], wbf[:, c * O:(c + 2) * O]
                        )
                for lb in range(SLB):
                    nc.scalar.dma_start(
                        xts[lb][:, PB * 128:QL], x1r[lb][:, PB * 128:QL]
                    )
                if first:
                    for c in range(17, NBF, 4):
                        ce = min(c + 4, NBF)
                        nc.sync.dma_start(
                            wbft[:, c * O:ce * O], wbf[:, c * O:ce * O]
                        )
                for q in range(1, 4):
                    for lb in range(SLB):
                        nc.scalar.dma_start(
                            xts[lb][:, q * QL:(q + 1) * QL],
                            x1r[lb][:, q * QL:(q + 1) * QL],
                        )
                if first:
                    if NFP8:
                        hw8 = NFP8 * O // 2
                        nc.sync.dma_start(wf8t[:, 0:hw8], wf8[:, 0:hw8])
                        nc.sync.dma_start(wf8t[:, hw8:], wf8[:, hw8:])
                    # x2 tails (needed only from lb=SLB on) go LAST so
                    # their 3 MB doesn't steal HBM bandwidth from the W
                    # stream during startup
                    nc.scalar.dma_start(
                        x2r[:, :, SLB * LB:L], x2s[:, :, SLB * LB:L]
                    )

                # ---- DVE: startup outer products ----
                # One broadcast mul covers a whole run of chunks sharing the
                # same g (x2 slice): in1 = x2[128,128] broadcast over chunks.
                def bcast_mul(xt_tile, t8_tile, lsl, c0, c1):
                    """outer products for chunks [c0, c1) of one l-block."""
                    if ABLATE == "nomul":
                        return
                    for g in range(GPC):
                        glo, ghi = max(c0, g * 32), min(c1, (g + 1) * 32, NBF)
                        if glo < ghi:
                            seg = xt_tile[:, glo * 128:ghi * 128].rearrange(
                                "p (c l) -> p c l", c=ghi - glo
                            )
                            bc = x2r[:, g, lsl].rearrange(
                                "p (o l) -> p o l", o=1
                            ).to_broadcast([128, ghi - glo, LB])
                            nc.vector.tensor_mul(seg, seg, bc)
                    if NFP8:
                        flo, fhi = max(c0, NBF), c1
                        if flo < fhi:
                            g = flo >> 5
                            seg8 = t8_tile[
                                :, (flo - NBF) * 128:(fhi - NBF) * 128
                            ].rearrange("p (c l) -> p c l", c=fhi - flo)
                            src = xt_tile[:, flo * 128:fhi * 128].rearrange(
                                "p (c l) -> p c l", c=fhi - flo
                            )
                            bc = x2r[:, g, lsl].rearrange(
                                "p (o l) -> p o l", o=1
                            ).to_broadcast([128, fhi - flo, LB])
                            nc.vector.tensor_mul(seg8, src, bc)

                # DVE mul order mirrors DMA arrival (the DVE queue is
                # in-order): chunk-0 for all l-blocks, rest of piece-a,
                # each piece-b half, then quarters 1..3.
                def q0_sl(lb):
                    return slice(lb * LB, (lb + 1) * LB)

                for lb in range(SLB):
                    bcast_mul(xts[lb], t8s[lb], q0_sl(lb), 0, PA)
                for lb in range(SLB):
                    bcast_mul(xts[lb], t8s[lb], q0_sl(lb), PA, PB)
                for lb in range(SLB):
                    bcast_mul(xts[lb], t8s[lb], q0_sl(lb), PB, 16)
                for q in range(1, 4):
                    for lb in range(SLB):
                        bcast_mul(xts[lb], t8s[lb], q0_sl(lb), q * 16, (q + 1) * 16)

                # ---- PE: startup, chunk-major across 8 psum banks ----
                if ABLATE == "nope":
                    # DMA+DVE only: emit steady slab loads + muls, no PE/out
                    for lb in range(SLB, NLB):
                        xt = xpool.tile([128, KL], bf, name="xt", tag="xt")
                        nc.scalar.dma_start(xt[:], x1r[lb])
                        t8 = (
                            t8pool.tile(
                                [128, NFP8 * 128], f8, name="t8", tag="t8"
                            )
                            if NFP8 else None
                        )
                        bcast_mul(
                            xt, t8, slice(lb * LB, (lb + 1) * LB), 0, NCHUNK
                        )
                    return
                # PE warm-up: self-contained dummy matmuls on a memset
                # scratch tile while the first W chunk + slab pieces are
                # still in flight, so the PE exits its low/mid p-states
                # (427ns/instr for the first ~3us otherwise) before real
                # work arrives.  The warm psum tile shares a bank with a
                # later pss tile; WAW ordering covers it.
                if first and ABLATE != "nope":
                    warm = wupool.tile([128, 128], bf, name="warm", tag="warm")
                    nc.vector.memset(warm[:], 0.0)
                    psw = psum.tile([128, 512], f32, name="pss", tag="ps")
                    for _ in range(int(os.environ.get("GL_WARM", "100"))):
                        nc.tensor.matmul(
                            psw[:, 0:128], warm[:], warm[:],
                            start=True, stop=True,
                        )

                pss = [
                    psum.tile([128, 512], f32, name="pss", tag="ps")
                    for _ in range(2 * SLB)
                ]

                def su_mm(c, lb, h):
                    nc.tensor.matmul(
                        pss[lb * 2 + h][:],
                        xts[lb][:, c * 128:(c + 1) * 128],
                        wbft[:, c * O + h * 512:c * O + h * 512 + 512],
                        start=(c == 0),
                        stop=(not use_f8 and c == NBF - 1),
                    )

                # chunk-major across all startup l-blocks: the W stream
                # supplies ~1 chunk per 0.6us dispatch slot while the PE
                # consumes one per 1.73us -- consuming any faster (e.g. a
                # single-lb bridge) just starves on W.  h in the middle:
                # each chunk's h0 matmuls only need the chunk's first 512
                # W columns (c0/c1 are DMA'd as 512-col halves).
                for c in range(NBF):
                    for h in range(2):
                        for lb in range(SLB):
                            su_mm(c, lb, h)
                for p in range(NFP8 // 2 if use_f8 else 0):
                    for lb in range(SLB):
                        for h in range(2):
                            nc.tensor.matmul(
                                pss[lb * 2 + h][:],
                                dr_lhs(t8s[lb], p),
                                wf8r[:, 2 * p:2 * p + 2, h * 512:h * 512 + 512],
                                start=False,
                                stop=(p == NFP8 // 2 - 1),
                                perf_mode=DR,
                            )
                for lb in range(SLB):
                    lsl = slice(lb * LB, (lb + 1) * LB)
                    ot = opool.tile([128, O], bf, name="ot", tag="ot")
                    for h in range(2):
                        hs = slice(h * 512, h * 512 + 512)
                        nc.scalar.mul(ot[:, hs], pss[lb * 2 + h][:], 1.0 / WSCALE)
                        nc.sync.dma_start(out[lsl, hs], ot[:, hs])

                # ---- steady state: lblock-major ----
                for lb in range(SLB, NLB):
                    last = lb == NLB - 1
                    if ABLATE == "nodma":
                        xt, t8 = xts[lb % SLB], t8s[lb % SLB]
                    else:
                        xt = xpool.tile([128, KL], bf, name="xt", tag="xt")
                        nc.scalar.dma_start(xt[:], x1r[lb])
                        t8 = (
                            t8pool.tile([128, NFP8 * 128], f8, name="t8", tag="t8")
                            if use_f8 else None
                        )
                    lsl = slice(lb * LB, (lb + 1) * LB)
                    if ABLATE != "nodma":
                        bcast_mul(xt, t8, lsl, 0, NCHUNK)
                    ot = opool.tile([128, O], bf, name="ot", tag="ot")
                    for h in range(2):
                        hs = slice(h * 512, h * 512 + 512)
                        ps = psum.tile([128, 512], f32, name="pss", tag="ps")
                        for c in range(NBF):
                            nc.tensor.matmul(
                                ps[:],
                                xt[:, c * 128:(c + 1) * 128],
                                wbft[:, c * O + h * 512:c * O + h * 512 + 512],
                                start=(c == 0),
                                stop=(not use_f8 and c == NBF - 1),
                            )
                        for p in range(NFP8 // 2 if use_f8 else 0):
                            nc.tensor.matmul(
                                ps[:],
                                dr_lhs(t8, p),
                                wf8r[:, 2 * p:2 * p + 2, h * 512:h * 512 + 512],
                                start=False,
                                stop=(p == NFP8 // 2 - 1),
                                perf_mode=DR,
                            )
                        nc.scalar.mul(ot[:, hs], ps[:], 1.0 / WSCALE)
                        if last and h == 1:
                            # final transfer: halve the tail across the
                            # two HWDGE queues (finer splits are fixed-
                            # latency dominated and slower)
                            l0 = lb * LB
                            nc.sync.dma_start(out[l0:l0 + 64, hs], ot[0:64, hs])
                            nc.scalar.dma_start(
                                out[l0 + 64:l0 + 128, hs], ot[64:128, hs]
                            )
                        else:
                            nc.sync.dma_start(out[lsl, hs], ot[:, hs])

            for _u in range(unroll):
                emit_body(_u == 0)

    nc.compile()
    return nc


def _prep_inputs(input1, input2, W):
    """Host-side shard + layout (transposes / gathers / dtype casts only)."""
    x1 = np.ascontiguousarray(input1, dtype=np.float32)
    x2 = np.ascontiguousarray(input2, dtype=np.float32)
    Wt = np.ascontiguousarray(W.T, dtype=np.float32) * np.float32(WSCALE)

    in_maps = []
    for core in range(NCORES):
        ks = slice(core * KL, (core + 1) * KL)
        gs = slice(core * GPC, (core + 1) * GPC)
        # weights: [k_local, o] -> [c, p, o] -> [p, c, o] (chunk-major free dim)
        wchunks = (
            Wt[ks].reshape(NCHUNK, 128, O).transpose(1, 0, 2)
        )  # [128, NCHUNK, O] fp32 (scaled)
        wbf = np.ascontiguousarray(
            wchunks[:, :NBF, :].reshape(128, NBF * O).astype(BF16)
        )
        wf8 = np.ascontiguousarray(
            wchunks[:, NBF:, :].reshape(128, NFP8 * O).astype(F8E4)
        )
        # x1 replicated over j: k_local = g*B*B + i*B + j -> x1[l, g, i]
        x1g = x1.reshape(L, G, B)[:, gs, :].transpose(1, 2, 0)  # [g, i, l]
        rep = np.repeat(x1g, B, axis=1).reshape(KL, L)          # [k_local, l]
        x1rc = (
            rep.reshape(NCHUNK, 128, NLB, LB)
            .transpose(2, 1, 0, 3)
            .reshape(NLB, 128, KL)
            .astype(BF16)
        )
        # x2 stacked twice along partitions: row p -> j = p % 64
        x2g = x2.reshape(L, G, B)[:, gs, :].transpose(1, 2, 0)  # [g, j, l]
        x2st = np.concatenate([x2g, x2g], axis=1).astype(BF16)  # [g, 128, l]
        x2st = x2st.transpose(1, 0, 2)                          # [128, g, l]
        m = {
            "wbf": wbf,
            "x1r": np.ascontiguousarray(x1rc),
            "x2s": np.ascontiguousarray(x2st),
        }
        if NFP8:
            m["wf8"] = wf8
        in_maps.append(m)
    return in_maps


def run(input1, input2, W, b, trace=False):
    """Shard, run on 8 NeuronCores, unshard. Returns (out, BassKernelResults)."""
    from concourse.bass_utils import run_bass_kernel_spmd

    if "nc" not in _cache:
        _cache["nc"] = _build_nc()
    nc = _cache["nc"]

    in_maps = _prep_inputs(input1, input2, W)
    res = run_bass_kernel_spmd(
        nc, in_maps, list(range(NCORES)), trace=trace
    )
    acc = np.zeros((L, O), dtype=np.float32)
    for core in range(NCORES):
        acc += res.results[core]["out"]
    acc += np.asarray(b, dtype=np.float32)[None, :]
    return acc, res


def kernel(input1, input2, W, b):
    out, _ = run(input1, input2, W, b, trace=False)
    return out


if __name__ == "__main__":
    rng = np.random.default_rng(0)
    x1 = rng.standard_normal((L, H), dtype=np.float32)
    x2 = rng.standard_normal((L, H), dtype=np.float32)
    W = rng.standard_normal((O, H * B), dtype=np.float32) / 256.0
    b = rng.standard_normal((O,), dtype=np.float32) / 256.0
    out = kernel(x1, x2, W, b)
    print("out", out.shape, out.dtype, float(np.abs(out).max()))
